# revision 26
# baseline (speedup 1.0000x reference)
"""Trainium2 Bass kernel for nn_BertWordPair (ragged RoPE pair scores).

Strategy (v3)
-------------
Inputs: qw, kw (B=8, S=768, H=4, D=256) fp32; token_index, thread_id (S,) int32.
Output: (B, S, S, H) fp32 where each (row-block, col-block) pair of the 6x128
thread-block grid uses one of three RoPE sign regimes:
    pp: rope(q,+pos) . rope(k,+pos)
    np: rope(q,-pos) . rope(k,+pos)   (0 < ti_r < ti_c)
    pn: rope(q,+pos) . rope(k,-pos)   (ti_c > 0, ti_r > ti_c)

Batch is sharded across the 8 cores (1 dialogue per core). Host precomputes
the +rotated variants q+, k+ in a de-interleaved (pair-index, token) fp16
layout; q-/k- are derived on device by DVE fp16 rotations
(x- = R(-2theta) x+, one 128-token cos2/sin2 table). Scores are fp16 matmuls
into fp32 PSUM, evacuated (ACT/Pool/DVE) to fp16 stages, streamed out on one
DMA ring (~8.0 MB @ 360 GB/s ~= 22.1 us).

Scheduling tricks:
- Token blocks are stored in DRAM/SBUF in permuted order [1,2,3,4,0,5] so
  the rotation-source blocks 1-4 arrive first and the DVE rotation chain
  (8 x ~1.85 us, the pacing constraint) starts at ~2.6 us.
- Output columns use the same permuted block order (host un-permutes), so
  each half-row's column set is {1,2,3} or {4,0,5}; half-rows are emitted in
  dependency-readiness order (row 0 needs no rotations but the last input
  chunk, so it fills middle slots).
- Dummy matmuls on the rotation table keep the PE p-state ramped while real
  matmul operands are still in flight.
"""

import os

import numpy as np

ROPE_BASE = 10000.0
B, S, H, D = 8, 768, 4, 256
HALF = D // 2  # 128
BLK = 128
NB = S // BLK  # 6
N_CORES = 8

_prog_cache = {}


def _host_rotations(qw, kw, token_index):
    """Return u/v (even/odd) +rotated and -rotated variants, fp32."""
    inv_freq = np.power(
        np.float32(ROPE_BASE),
        (np.arange(HALF, dtype=np.float32) * np.float32(-2.0 / D)),
    )  # (HALF,)
    pos = token_index.astype(np.float32)  # (S,)
    theta = pos[:, None] * inv_freq[None, :]  # (S, HALF)
    cos = np.cos(theta)[None, :, None, :]  # (1,S,1,HALF)
    sin = np.sin(theta)[None, :, None, :]

    out = []
    for x in (qw, kw):
        u = x[..., 0::2]  # (B,S,H,HALF)
        v = x[..., 1::2]
        uc = u * cos
        vs = v * sin
        vc = v * cos
        us = u * sin
        out.append((uc - vs, vc + us))  # positive rotation
        out.append((uc + vs, vc - us))  # negative rotation
    return out  # [(qp_u,qp_v),(qn_u,qn_v),(kp_u,kp_v),(kn_u,kn_v)]


def _to_device_layout(u, v, blocks):
    """(B,S,H,HALF) u/v -> (B, H, 2, HALF, T) fp16 for the given token blocks."""
    cols = np.concatenate([np.arange(b * BLK, (b + 1) * BLK) for b in blocks])
    u = u[:, cols]  # (B,T,H,HALF)
    v = v[:, cols]
    arr = np.stack([u, v], axis=2)  # (B,T,2,H,HALF)
    arr = np.transpose(arr, (0, 3, 2, 4, 1))  # (B,H,2,HALF,T)
    return np.ascontiguousarray(arr.astype(np.float16))


def _regime_map(thread_id):
    """Return (regimes, ok). regimes[i][j] in {'pp','np','pn'} per 128-block."""
    tid = np.asarray(thread_id)
    if tid.shape[0] != S:
        return None, False
    blocks = tid.reshape(NB, BLK)
    if not np.all(blocks == blocks[:, :1]):
        return None, False
    tvals = blocks[:, 0]
    regimes = []
    for i in range(NB):
        row = []
        for j in range(NB):
            ti_r, ti_c = tvals[i], tvals[j]
            if ti_r > 0 and ti_r < ti_c:
                row.append("np")
            elif ti_c > 0 and ti_r > ti_c:
                row.append("pn")
            else:
                row.append("pp")
        regimes.append(row)
    return regimes, True


# ---------------------------------------------------------------------------
# Schedule configuration (tuned against the cost-model simulator).
# ---------------------------------------------------------------------------
DEFAULT_CFG = {
    # token/column block order in DRAM+SBUF (rotation sources first)
    "perm": [1, 2, 3, 4, 0, 5],
    # input DMA chunks over permuted positions [lo, hi), both d-chunks each
    "input_order": [
        ("kp", 0, 2),
        ("kt",),
        ("qp", 0, 2),
        ("kp", 2, 4),
        ("qp", 2, 4),
        ("kp", 4, 6),
        ("qp", 4, 6),
    ],
    # DVE rotation unit order; ("kn", r) = r'th block of kn run
    "rot_order": [
        ("kn", 0),
        ("qn", 0),
        ("kn", 1),
        ("qn", 1),
        ("kn", 2),
        ("qn", 2),
        ("kn", 3),
        ("qn", 3),
    ],
    # output pair groups (row, pairidx) in emission order; pair p covers
    # permuted col positions {2p, 2p+1}; one 2-bank evac + one DMA each
    "groups": [
        (2, 0),
        (1, 0),
        (1, 1),
        (3, 0),
        (4, 0),
        (1, 2),
        (0, 0),
        (0, 1),
        (0, 2),
        (5, 0),
        (5, 2),
        (2, 1),
        (2, 2),
        (4, 1),
        (3, 1),
        (3, 2),
        (5, 1),
        (4, 2),
    ],
    # evac engine per group: A=ACT, P=Pool, V=DVE
    "evac": "APAAPAPAAPAPAPVAVP",
    # rotation units whose final add is folded into the consuming matmuls
    # (4-product accumulation); only sensible for fanout-1 units
    "skip_add": [("kn", 3), ("qn", 3)],
    # PE warmup dummy matmuls before real work (the cost model runs
    # dependency-paced matmuls at full speed, so 0 is best)
    "warmup": 0,
}


def _build_program(regimes, qn_blocks, kn_blocks, cfg):
    import concourse.bass as bass  # noqa: F401
    import concourse.tile as tile
    from concourse import bacc, mybir

    f16 = mybir.dt.float16
    f32 = mybir.dt.float32

    perm = cfg["perm"]
    bpos = {b: i for i, b in enumerate(perm)}
    nqn = len(qn_blocks)
    nkn = len(kn_blocks)
    qn_pos = {b: idx for idx, b in enumerate(qn_blocks)}
    kn_pos = {b: idx for idx, b in enumerate(kn_blocks)}

    nc = bacc.Bacc(None, target_bir_lowering=False)
    qp_d = nc.dram_tensor("qp", [H, 2, HALF, S], f16, kind="ExternalInput")
    kp_d = nc.dram_tensor("kp", [H, 2, HALF, S], f16, kind="ExternalInput")
    # [c2 | s2 | -s2 | c2] over one 128-token period (all rotated blocks share
    # one token pattern; checked on host). tabA=[c2|s2], tabB=[-s2|c2].
    kt_d = nc.dram_tensor("kt", [HALF, 4 * BLK], f16, kind="ExternalInput")
    out_d = nc.dram_tensor("out", [S, S, H], f16, kind="ExternalOutput")

    with tile.TileContext(nc) as tc:
        with (
            tc.tile_pool(name="inp", bufs=1) as inp,
            tc.tile_pool(name="psum", bufs=4, space="PSUM") as pp,
            tc.tile_pool(name="stage", bufs=6) as stp,
            tc.tile_pool(name="rtmp", bufs=4) as rtmp,
        ):
            qp_t = inp.tile([HALF, H * 2 * S], f16, tag="qp")
            kp_t = inp.tile([HALF, H * 2 * S], f16, tag="kp")
            qn_t = inp.tile([HALF, H * 2 * nqn * BLK], f16, tag="qn")
            kn_t = inp.tile([HALF, H * 2 * nkn * BLK], f16, tag="kn")
            kt_t = inp.tile([HALF, 4 * BLK], f16, tag="kt")

            qp_v = qp_t[:].rearrange("p (h c t) -> p h c t", h=H, c=2, t=S)
            kp_v = kp_t[:].rearrange("p (h c t) -> p h c t", h=H, c=2, t=S)
            qp_dv = qp_d[:].rearrange("h c p t -> p h c t")
            kp_dv = kp_d[:].rearrange("h c p t -> p h c t")

            for chunk in cfg["input_order"]:
                if chunk[0] == "kt":
                    nc.sync.dma_start(kt_t[:], kt_d[:])
                else:
                    _, lo, hi = chunk
                    tv = qp_v if chunk[0] == "qp" else kp_v
                    dv = qp_dv if chunk[0] == "qp" else kp_dv
                    nc.sync.dma_start(
                        tv[:, :, :, lo * BLK : hi * BLK],
                        dv[:, :, :, lo * BLK : hi * BLK],
                    )

            # PE warmup: dummy matmuls on the table keep the tensor engine's
            # p-state ramped while real operands stream in. They write group
            # 0's bank tile, whose real matmuls overwrite it afterwards.
            warm = pp.tile([BLK, 2 * BLK * H], f32, tag="bank")
            for w in range(cfg["warmup"]):
                nc.tensor.matmul(
                    warm[:, (w % 8) * BLK : (w % 8) * BLK + BLK],
                    kp_t[:, 0:BLK],
                    kp_t[:, BLK : 2 * BLK],
                    start=True,
                    stop=True,
                )

            tabA = kt_t[:, 0 : 2 * BLK].rearrange("p (c t) -> p c t", c=2)
            tabB = kt_t[:, 2 * BLK : 4 * BLK].rearrange("p (c t) -> p c t", c=2)
            tabA4 = tabA.copy()
            tabA4.ap = tabA4.ap[:1] + [[0, H]] + tabA4.ap[1:]
            tabB4 = tabB.copy()
            tabB4.ap = tabB4.ap[:1] + [[0, H]] + tabB4.ap[1:]

            skip_add = {tuple(u) for u in cfg.get("skip_add", [])}
            zsaved = {}

            def emit_rotation(unit):
                # x- = R(-2theta) x+ for one 128-token block, all heads:
                #   u- = u*c2 + v*s2 ; v- = v*c2 - u*s2
                # X  = (u,v)*[c2|s2]  -> u- = X.lo  + X.hi
                # Y' = (u,v)*[-s2|c2] -> v- = Y'.lo + Y'.hi
                # One fused add produces both: dst[c=xy] = Z[xy].lo + Z[xy].hi
                # For skip_add units the add is folded into the consuming
                # matmuls (4-product accumulation), so only Z is produced.
                kind, ridx = unit
                src_t = qp_t if kind == "qn" else kp_t
                dst_t = qn_t if kind == "qn" else kn_t
                b = (qn_blocks if kind == "qn" else kn_blocks)[ridx]
                nrun = nqn if kind == "qn" else nkn
                p0 = bpos[b] * BLK
                src = (
                    src_t[:]
                    .rearrange("p (h c t) -> p h c t", h=H, c=2, t=S)[
                        :, :, :, p0 : p0 + BLK
                    ]
                )  # (p, h, 2, 128)
                if unit in skip_add:
                    z = rtmp.tile([HALF, 2 * H * 2 * BLK], f16, tag="zkeep")
                else:
                    z = rtmp.tile([HALF, 2 * H * 2 * BLK], f16, tag="z")
                z_v = z[:].rearrange("p (xy h c t) -> p xy h c t", xy=2, h=H, c=2)
                nc.vector.tensor_mul(z_v[:, 0], src, tabA4)
                nc.vector.tensor_mul(z_v[:, 1], src, tabB4)
                if unit in skip_add:
                    zsaved[unit] = z_v
                    return
                dst = (
                    dst_t[:]
                    .rearrange("p (h c t) -> p c h t", h=H, c=2, t=nrun * BLK)[
                        :, :, :, ridx * BLK : (ridx + 1) * BLK
                    ]
                )  # (p, c, h, 128) with c outermost
                nc.vector.tensor_add(dst, z_v[:, :, :, 0], z_v[:, :, :, 1])

            for unit in cfg["rot_order"]:
                kind, ridx = unit
                if ridx < (nqn if kind == "qn" else nkn):
                    emit_rotation(unit)

            def lhs_slice(variant, h, c, blk):
                if variant == "p":
                    return qp_t[:, (h * 2 + c) * S + bpos[blk] * BLK :][:, :BLK]
                return qn_t[:, (h * 2 + c) * (nqn * BLK) + qn_pos[blk] * BLK :][:, :BLK]

            def rhs_slice(variant, h, c, blk):
                if variant == "p":
                    return kp_t[:, (h * 2 + c) * S + bpos[blk] * BLK :][:, :BLK]
                return kn_t[:, (h * 2 + c) * (nkn * BLK) + kn_pos[blk] * BLK :][:, :BLK]

            for gidx, (i, pair) in enumerate(cfg["groups"]):
                stage = stp.tile([BLK, 2 * BLK * H], f16, tag="pair")
                bank2 = warm if gidx == 0 else pp.tile(
                    [BLK, 2 * BLK * H], f32, tag="bank"
                )
                for idx in range(2):
                    jj = pair * 2 + idx
                    j = perm[jj]
                    reg = regimes[i][j]
                    qv = "n" if reg == "np" else "p"
                    kv = "n" if reg == "pn" else "p"
                    zq = zsaved.get(("qn", qn_pos.get(i))) if reg == "np" else None
                    zk = zsaved.get(("kn", kn_pos.get(j))) if reg == "pn" else None
                    for h in range(H):
                        if zq is not None:
                            # qn_i folded: qn_e = X.lo+X.hi, qn_o = Y.lo+Y.hi
                            mms = [
                                (zq[:, 0, h, 0], rhs_slice("p", h, 0, j)),
                                (zq[:, 0, h, 1], rhs_slice("p", h, 0, j)),
                                (zq[:, 1, h, 0], rhs_slice("p", h, 1, j)),
                                (zq[:, 1, h, 1], rhs_slice("p", h, 1, j)),
                            ]
                        elif zk is not None:
                            mms = [
                                (lhs_slice("p", h, 0, i), zk[:, 0, h, 0]),
                                (lhs_slice("p", h, 0, i), zk[:, 0, h, 1]),
                                (lhs_slice("p", h, 1, i), zk[:, 1, h, 0]),
                                (lhs_slice("p", h, 1, i), zk[:, 1, h, 1]),
                            ]
                        else:
                            mms = [
                                (lhs_slice(qv, h, 0, i), rhs_slice(kv, h, 0, j)),
                                (lhs_slice(qv, h, 1, i), rhs_slice(kv, h, 1, j)),
                            ]
                        for m, (lhs, rhs) in enumerate(mms):
                            nc.tensor.matmul(
                                bank2[
                                    :,
                                    (idx * H + h) * BLK : (idx * H + h + 1) * BLK,
                                ],
                                lhs,
                                rhs,
                                start=(h == 0 and m == 0),
                                stop=(h == H - 1 and m == len(mms) - 1),
                            )
                # one 2-bank head-interleaving evac: (p, (jj h n)) fp32 ->
                # (p, (jj n h)) fp16
                dst_blk = stage[:].rearrange("p (jj n h) -> p jj n h", jj=2, h=H)
                src_blk = bank2[:].rearrange("p (jj h n) -> p jj n h", jj=2, h=H)
                eng = cfg["evac"][gidx]
                if eng == "A":
                    nc.scalar.copy(dst_blk, src_blk)
                elif eng == "P":
                    nc.gpsimd.tensor_copy(dst_blk, src_blk)
                else:
                    nc.vector.tensor_copy(dst_blk, src_blk)
                nc.sync.dma_start(
                    out_d[
                        i * BLK : (i + 1) * BLK,
                        pair * (2 * BLK) : (pair + 1) * (2 * BLK),
                    ].rearrange("p n h -> p (n h)"),
                    stage[:],
                )
    nc.finalize()
    return nc


def _reference_fallback(qw, kw, token_index, thread_id):
    """Pure numpy fallback for unexpected block structure."""
    rots = _host_rotations(qw, kw, token_index)
    (qp_u, qp_v), (qn_u, qn_v), (kp_u, kp_v), (kn_u, kn_v) = rots

    def interleave(u, v):
        x = np.empty(u.shape[:-1] + (D,), dtype=np.float32)
        x[..., 0::2] = u
        x[..., 1::2] = v
        return x

    q_p = interleave(qp_u, qp_v)
    q_n = interleave(qn_u, qn_v)
    k_p = interleave(kp_u, kp_v)
    k_n = interleave(kn_u, kn_v)
    s_pp = np.einsum("bmhd,bnhd->bmnh", q_p, k_p)
    s_np = np.einsum("bmhd,bnhd->bmnh", q_n, k_p)
    s_pn = np.einsum("bmhd,bnhd->bmnh", q_p, k_n)
    ti_r = thread_id[:, None]
    ti_c = thread_id[None, :]
    sx = ((ti_r > 0) & (ti_r < ti_c))[None, :, :, None]
    sy = ((ti_c > 0) & (ti_r > ti_c))[None, :, :, None]
    return np.where(sx, s_np, np.where(sy, s_pn, s_pp)).astype(np.float32)


def kernel(qw, kw, token_index, thread_id, _cfg=None):
    qw = np.asarray(qw, dtype=np.float32)
    kw = np.asarray(kw, dtype=np.float32)
    token_index = np.asarray(token_index)
    thread_id = np.asarray(thread_id)
    cfg = _cfg or DEFAULT_CFG

    regimes, ok = _regime_map(thread_id)
    if (
        not ok
        or qw.shape != (B, S, H, D)
        or kw.shape != (B, S, H, D)
        or token_index.shape != (S,)
    ):
        return _reference_fallback(qw, kw, token_index, thread_id)

    qn_blocks = sorted(
        {i for i in range(NB) if any(regimes[i][j] == "np" for j in range(NB))}
    )
    kn_blocks = sorted(
        {j for j in range(NB) if any(regimes[i][j] == "pn" for i in range(NB))}
    )
    if not qn_blocks:
        qn_blocks = [0]
    if not kn_blocks:
        kn_blocks = [0]

    # Device rotation requires every rotated block to share one 128-token
    # index pattern (the [c2|s2|c2] table covers a single period).
    tok_blocks = token_index.reshape(NB, BLK)
    rot_blocks = sorted(set(qn_blocks) | set(kn_blocks))
    uniform = all(
        np.array_equal(tok_blocks[b], tok_blocks[rot_blocks[0]]) for b in rot_blocks
    )
    if not uniform:
        return _reference_fallback(qw, kw, token_index, thread_id)

    rots = _host_rotations(qw, kw, token_index)
    (qp_u, qp_v), (qn_u, qn_v), (kp_u, kp_v), (kn_u, kn_v) = rots
    perm = cfg["perm"]
    qp_a = _to_device_layout(qp_u, qp_v, perm)  # (B,H,2,HALF,S) permuted blocks
    kp_a = _to_device_layout(kp_u, kp_v, perm)

    inv_freq = np.power(
        np.float32(ROPE_BASE),
        (np.arange(HALF, dtype=np.float32) * np.float32(-2.0 / D)),
    )
    theta = (
        token_index[rot_blocks[0] * BLK : (rot_blocks[0] + 1) * BLK]
        .astype(np.float32)[:, None]
        * inv_freq[None, :]
    )
    c2 = np.cos(2.0 * theta).T  # (HALF, BLK)
    s2 = np.sin(2.0 * theta).T
    kt_a = np.ascontiguousarray(
        np.concatenate([c2, s2, -s2, c2], axis=1).astype(np.float16)
    )

    key = (
        tuple(tuple(r) for r in regimes),
        tuple(qn_blocks),
        tuple(kn_blocks),
        str(cfg),
    )
    if key not in _prog_cache:
        _prog_cache[key] = _build_program(regimes, qn_blocks, kn_blocks, cfg)
    nc = _prog_cache[key]

    from concourse.bass_utils import run_bass_kernel_spmd

    in_maps = [{"qp": qp_a[b], "kp": kp_a[b], "kt": kt_a} for b in range(B)]
    trace = bool(int(os.environ.get("KERNEL_TRACE", "0")))
    res = None
    for attempt in range(3):
        try:
            res = run_bass_kernel_spmd(
                nc,
                in_maps,
                core_ids=list(range(N_CORES)),
                trace=trace,
            )
            break
        except Exception:
            # transient NRT/device blips have been observed on otherwise-
            # correct programs; retry.
            if attempt == 2:
                raise
    if res.exec_time_ns is not None:
        print(f"HW exec time: {res.exec_time_ns} ns")
    if res.instructions_and_trace is not None:
        print(f"trace: {res.instructions_and_trace[1]}")

    out_dev = np.stack([res.results[b]["out"] for b in range(B)], axis=0)
    # un-permute output columns: device col position k holds natural block
    # perm[k]
    out = np.empty_like(out_dev)
    for k, b in enumerate(perm):
        out[:, :, b * BLK : (b + 1) * BLK] = out_dev[:, :, k * BLK : (k + 1) * BLK]
    return out.astype(np.float32)


# revision 27
# speedup vs baseline: 1.0111x; 1.0111x over previous
"""Trainium2 Bass kernel for nn_BertWordPair (ragged RoPE pair scores).

Strategy (v3)
-------------
Inputs: qw, kw (B=8, S=768, H=4, D=256) fp32; token_index, thread_id (S,) int32.
Output: (B, S, S, H) fp32 where each (row-block, col-block) pair of the 6x128
thread-block grid uses one of three RoPE sign regimes:
    pp: rope(q,+pos) . rope(k,+pos)
    np: rope(q,-pos) . rope(k,+pos)   (0 < ti_r < ti_c)
    pn: rope(q,+pos) . rope(k,-pos)   (ti_c > 0, ti_r > ti_c)

Batch is sharded across the 8 cores (1 dialogue per core). Host precomputes
the +rotated variants q+, k+ in a de-interleaved (pair-index, token) fp16
layout; q-/k- are derived on device by DVE fp16 rotations
(x- = R(-2theta) x+, one 128-token cos2/sin2 table). Scores are fp16 matmuls
into fp32 PSUM, evacuated (ACT/Pool/DVE) to fp16 stages, streamed out on one
DMA ring (~8.0 MB @ 360 GB/s ~= 22.1 us).

Scheduling tricks:
- Token blocks are stored in DRAM/SBUF in permuted order [1,2,3,4,0,5] so
  the rotation-source blocks 1-4 arrive first and the DVE rotation chain
  (8 x ~1.85 us, the pacing constraint) starts at ~2.6 us.
- Output columns use the same permuted block order (host un-permutes), so
  each half-row's column set is {1,2,3} or {4,0,5}; half-rows are emitted in
  dependency-readiness order (row 0 needs no rotations but the last input
  chunk, so it fills middle slots).
- Dummy matmuls on the rotation table keep the PE p-state ramped while real
  matmul operands are still in flight.
"""

import os

import numpy as np

ROPE_BASE = 10000.0
B, S, H, D = 8, 768, 4, 256
HALF = D // 2  # 128
BLK = 128
NB = S // BLK  # 6
N_CORES = 8

_prog_cache = {}


def _host_rotations(qw, kw, token_index):
    """Return u/v (even/odd) +rotated and -rotated variants, fp32."""
    inv_freq = np.power(
        np.float32(ROPE_BASE),
        (np.arange(HALF, dtype=np.float32) * np.float32(-2.0 / D)),
    )  # (HALF,)
    pos = token_index.astype(np.float32)  # (S,)
    theta = pos[:, None] * inv_freq[None, :]  # (S, HALF)
    cos = np.cos(theta)[None, :, None, :]  # (1,S,1,HALF)
    sin = np.sin(theta)[None, :, None, :]

    out = []
    for x in (qw, kw):
        u = x[..., 0::2]  # (B,S,H,HALF)
        v = x[..., 1::2]
        uc = u * cos
        vs = v * sin
        vc = v * cos
        us = u * sin
        out.append((uc - vs, vc + us))  # positive rotation
        out.append((uc + vs, vc - us))  # negative rotation
    return out  # [(qp_u,qp_v),(qn_u,qn_v),(kp_u,kp_v),(kn_u,kn_v)]


def _to_device_layout(u, v, blocks):
    """(B,S,H,HALF) u/v -> (B, H, 2, HALF, T) fp16 for the given token blocks."""
    cols = np.concatenate([np.arange(b * BLK, (b + 1) * BLK) for b in blocks])
    u = u[:, cols]  # (B,T,H,HALF)
    v = v[:, cols]
    arr = np.stack([u, v], axis=2)  # (B,T,2,H,HALF)
    arr = np.transpose(arr, (0, 3, 2, 4, 1))  # (B,H,2,HALF,T)
    return np.ascontiguousarray(arr.astype(np.float16))


def _regime_map(thread_id):
    """Return (regimes, ok). regimes[i][j] in {'pp','np','pn'} per 128-block."""
    tid = np.asarray(thread_id)
    if tid.shape[0] != S:
        return None, False
    blocks = tid.reshape(NB, BLK)
    if not np.all(blocks == blocks[:, :1]):
        return None, False
    tvals = blocks[:, 0]
    regimes = []
    for i in range(NB):
        row = []
        for j in range(NB):
            ti_r, ti_c = tvals[i], tvals[j]
            if ti_r > 0 and ti_r < ti_c:
                row.append("np")
            elif ti_c > 0 and ti_r > ti_c:
                row.append("pn")
            else:
                row.append("pp")
        regimes.append(row)
    return regimes, True


# ---------------------------------------------------------------------------
# Schedule configuration (tuned against the cost-model simulator).
# ---------------------------------------------------------------------------
DEFAULT_CFG = {
    # token/column block order in DRAM+SBUF (rotation sources first)
    "perm": [1, 2, 3, 4, 0, 5],
    # input DMA chunks over permuted positions [lo, hi), both d-chunks each
    "input_order": [
        ("kt",),
        ("kp", 0, 2),
        ("qp", 0, 2),
        ("kp", 2, 4),
        ("qp", 2, 4),
        ("kp", 4, 6),
        ("qp", 4, 6),
    ],
    # DVE rotation unit order; ("kn", r) = r'th block of kn run
    "rot_order": [
        ("kn", 0),
        ("qn", 0),
        ("kn", 1),
        ("qn", 1),
        ("kn", 2),
        ("qn", 2),
        ("kn", 3),
        ("qn", 3),
    ],
    # output pair groups (row, pairidx) in emission order; pair p covers
    # permuted col positions {2p, 2p+1}; one 2-bank evac + one DMA each
    "groups": [
        (2, 0),
        (1, 0),
        (1, 1),
        (3, 0),
        (4, 0),
        (1, 2),
        (0, 0),
        (0, 1),
        (0, 2),
        (5, 0),
        (5, 2),
        (2, 1),
        (2, 2),
        (4, 1),
        (3, 1),
        (3, 2),
        (5, 1),
        (4, 2),
    ],
    # evac engine per group: A=ACT, P=Pool, V=DVE
    "evac": "APAAPAPAAPAPAPVAVP",
    # rotation units whose final add is folded into the consuming matmuls
    # (4-product accumulation); only sensible for fanout-1 units
    "skip_add": [("kn", 3), ("qn", 3)],
    # PE warmup dummy matmuls before real work (the cost model runs
    # dependency-paced matmuls at full speed, so 0 is best)
    "warmup": 17,
}


def _build_program(regimes, qn_blocks, kn_blocks, cfg):
    import concourse.bass as bass  # noqa: F401
    import concourse.tile as tile
    from concourse import bacc, mybir

    f16 = mybir.dt.float16
    f32 = mybir.dt.float32

    perm = cfg["perm"]
    bpos = {b: i for i, b in enumerate(perm)}
    nqn = len(qn_blocks)
    nkn = len(kn_blocks)
    qn_pos = {b: idx for idx, b in enumerate(qn_blocks)}
    kn_pos = {b: idx for idx, b in enumerate(kn_blocks)}

    nc = bacc.Bacc(None, target_bir_lowering=False)
    qp_d = nc.dram_tensor("qp", [H, 2, HALF, S], f16, kind="ExternalInput")
    kp_d = nc.dram_tensor("kp", [H, 2, HALF, S], f16, kind="ExternalInput")
    # [c2 | s2 | -s2 | c2] over one 128-token period (all rotated blocks share
    # one token pattern; checked on host). tabA=[c2|s2], tabB=[-s2|c2].
    kt_d = nc.dram_tensor("kt", [HALF, 4 * BLK], f16, kind="ExternalInput")
    out_d = nc.dram_tensor("out", [S, S, H], f16, kind="ExternalOutput")

    with tile.TileContext(nc) as tc:
        with (
            tc.tile_pool(name="inp", bufs=1) as inp,
            tc.tile_pool(name="psum", bufs=4, space="PSUM") as pp,
            tc.tile_pool(name="stage", bufs=6) as stp,
            tc.tile_pool(name="rtmp", bufs=4) as rtmp,
        ):
            qp_t = inp.tile([HALF, H * 2 * S], f16, tag="qp")
            kp_t = inp.tile([HALF, H * 2 * S], f16, tag="kp")
            qn_t = inp.tile([HALF, H * 2 * nqn * BLK], f16, tag="qn")
            kn_t = inp.tile([HALF, H * 2 * nkn * BLK], f16, tag="kn")
            kt_t = inp.tile([HALF, 4 * BLK], f16, tag="kt")

            qp_v = qp_t[:].rearrange("p (h c t) -> p h c t", h=H, c=2, t=S)
            kp_v = kp_t[:].rearrange("p (h c t) -> p h c t", h=H, c=2, t=S)
            qp_dv = qp_d[:].rearrange("h c p t -> p h c t")
            kp_dv = kp_d[:].rearrange("h c p t -> p h c t")

            for chunk in cfg["input_order"]:
                if chunk[0] == "kt":
                    nc.sync.dma_start(kt_t[:], kt_d[:])
                else:
                    _, lo, hi = chunk
                    tv = qp_v if chunk[0] == "qp" else kp_v
                    dv = qp_dv if chunk[0] == "qp" else kp_dv
                    nc.sync.dma_start(
                        tv[:, :, :, lo * BLK : hi * BLK],
                        dv[:, :, :, lo * BLK : hi * BLK],
                    )

            # PE warmup: dummy matmuls on the table keep the tensor engine's
            # p-state ramped while real operands stream in. They write group
            # 0's bank tile, whose real matmuls overwrite it afterwards.
            warm = pp.tile([BLK, 2 * BLK * H], f32, tag="bank")
            for w in range(cfg["warmup"]):
                nc.tensor.matmul(
                    warm[:, (w % 8) * BLK : (w % 8) * BLK + BLK],
                    kt_t[:, 0:BLK],
                    kt_t[:, BLK : 2 * BLK],
                    start=True,
                    stop=True,
                )

            tabA = kt_t[:, 0 : 2 * BLK].rearrange("p (c t) -> p c t", c=2)
            tabB = kt_t[:, 2 * BLK : 4 * BLK].rearrange("p (c t) -> p c t", c=2)
            tabA4 = tabA.copy()
            tabA4.ap = tabA4.ap[:1] + [[0, H]] + tabA4.ap[1:]
            tabB4 = tabB.copy()
            tabB4.ap = tabB4.ap[:1] + [[0, H]] + tabB4.ap[1:]

            skip_add = {tuple(u) for u in cfg.get("skip_add", [])}
            zsaved = {}

            def emit_rotation(unit):
                # x- = R(-2theta) x+ for one 128-token block, all heads:
                #   u- = u*c2 + v*s2 ; v- = v*c2 - u*s2
                # X  = (u,v)*[c2|s2]  -> u- = X.lo  + X.hi
                # Y' = (u,v)*[-s2|c2] -> v- = Y'.lo + Y'.hi
                # One fused add produces both: dst[c=xy] = Z[xy].lo + Z[xy].hi
                # For skip_add units the add is folded into the consuming
                # matmuls (4-product accumulation), so only Z is produced.
                kind, ridx = unit
                src_t = qp_t if kind == "qn" else kp_t
                dst_t = qn_t if kind == "qn" else kn_t
                b = (qn_blocks if kind == "qn" else kn_blocks)[ridx]
                nrun = nqn if kind == "qn" else nkn
                p0 = bpos[b] * BLK
                src = (
                    src_t[:]
                    .rearrange("p (h c t) -> p h c t", h=H, c=2, t=S)[
                        :, :, :, p0 : p0 + BLK
                    ]
                )  # (p, h, 2, 128)
                if unit in skip_add:
                    z = rtmp.tile([HALF, 2 * H * 2 * BLK], f16, tag="zkeep")
                else:
                    z = rtmp.tile([HALF, 2 * H * 2 * BLK], f16, tag="z")
                z_v = z[:].rearrange("p (xy h c t) -> p xy h c t", xy=2, h=H, c=2)
                nc.vector.tensor_mul(z_v[:, 0], src, tabA4)
                nc.vector.tensor_mul(z_v[:, 1], src, tabB4)
                if unit in skip_add:
                    zsaved[unit] = z_v
                    return
                dst = (
                    dst_t[:]
                    .rearrange("p (h c t) -> p c h t", h=H, c=2, t=nrun * BLK)[
                        :, :, :, ridx * BLK : (ridx + 1) * BLK
                    ]
                )  # (p, c, h, 128) with c outermost
                nc.vector.tensor_add(dst, z_v[:, :, :, 0], z_v[:, :, :, 1])

            for unit in cfg["rot_order"]:
                kind, ridx = unit
                if ridx < (nqn if kind == "qn" else nkn):
                    emit_rotation(unit)

            def lhs_slice(variant, h, c, blk):
                if variant == "p":
                    return qp_t[:, (h * 2 + c) * S + bpos[blk] * BLK :][:, :BLK]
                return qn_t[:, (h * 2 + c) * (nqn * BLK) + qn_pos[blk] * BLK :][:, :BLK]

            def rhs_slice(variant, h, c, blk):
                if variant == "p":
                    return kp_t[:, (h * 2 + c) * S + bpos[blk] * BLK :][:, :BLK]
                return kn_t[:, (h * 2 + c) * (nkn * BLK) + kn_pos[blk] * BLK :][:, :BLK]

            for gidx, (i, pair) in enumerate(cfg["groups"]):
                stage = stp.tile([BLK, 2 * BLK * H], f16, tag="pair")
                bank2 = warm if gidx == 0 else pp.tile(
                    [BLK, 2 * BLK * H], f32, tag="bank"
                )
                for idx in range(2):
                    jj = pair * 2 + idx
                    j = perm[jj]
                    reg = regimes[i][j]
                    qv = "n" if reg == "np" else "p"
                    kv = "n" if reg == "pn" else "p"
                    zq = zsaved.get(("qn", qn_pos.get(i))) if reg == "np" else None
                    zk = zsaved.get(("kn", kn_pos.get(j))) if reg == "pn" else None
                    for h in range(H):
                        if zq is not None:
                            # qn_i folded: qn_e = X.lo+X.hi, qn_o = Y.lo+Y.hi
                            mms = [
                                (zq[:, 0, h, 0], rhs_slice("p", h, 0, j)),
                                (zq[:, 0, h, 1], rhs_slice("p", h, 0, j)),
                                (zq[:, 1, h, 0], rhs_slice("p", h, 1, j)),
                                (zq[:, 1, h, 1], rhs_slice("p", h, 1, j)),
                            ]
                        elif zk is not None:
                            mms = [
                                (lhs_slice("p", h, 0, i), zk[:, 0, h, 0]),
                                (lhs_slice("p", h, 0, i), zk[:, 0, h, 1]),
                                (lhs_slice("p", h, 1, i), zk[:, 1, h, 0]),
                                (lhs_slice("p", h, 1, i), zk[:, 1, h, 1]),
                            ]
                        else:
                            mms = [
                                (lhs_slice(qv, h, 0, i), rhs_slice(kv, h, 0, j)),
                                (lhs_slice(qv, h, 1, i), rhs_slice(kv, h, 1, j)),
                            ]
                        for m, (lhs, rhs) in enumerate(mms):
                            nc.tensor.matmul(
                                bank2[
                                    :,
                                    (idx * H + h) * BLK : (idx * H + h + 1) * BLK,
                                ],
                                lhs,
                                rhs,
                                start=(h == 0 and m == 0),
                                stop=(h == H - 1 and m == len(mms) - 1),
                            )
                # one 2-bank head-interleaving evac: (p, (jj h n)) fp32 ->
                # (p, (jj n h)) fp16
                dst_blk = stage[:].rearrange("p (jj n h) -> p jj n h", jj=2, h=H)
                src_blk = bank2[:].rearrange("p (jj h n) -> p jj n h", jj=2, h=H)
                eng = cfg["evac"][gidx]
                if eng == "A":
                    nc.scalar.copy(dst_blk, src_blk)
                elif eng == "P":
                    nc.gpsimd.tensor_copy(dst_blk, src_blk)
                else:
                    nc.vector.tensor_copy(dst_blk, src_blk)
                nc.sync.dma_start(
                    out_d[
                        i * BLK : (i + 1) * BLK,
                        pair * (2 * BLK) : (pair + 1) * (2 * BLK),
                    ].rearrange("p n h -> p (n h)"),
                    stage[:],
                )
    nc.finalize()
    return nc


def _reference_fallback(qw, kw, token_index, thread_id):
    """Pure numpy fallback for unexpected block structure."""
    rots = _host_rotations(qw, kw, token_index)
    (qp_u, qp_v), (qn_u, qn_v), (kp_u, kp_v), (kn_u, kn_v) = rots

    def interleave(u, v):
        x = np.empty(u.shape[:-1] + (D,), dtype=np.float32)
        x[..., 0::2] = u
        x[..., 1::2] = v
        return x

    q_p = interleave(qp_u, qp_v)
    q_n = interleave(qn_u, qn_v)
    k_p = interleave(kp_u, kp_v)
    k_n = interleave(kn_u, kn_v)
    s_pp = np.einsum("bmhd,bnhd->bmnh", q_p, k_p)
    s_np = np.einsum("bmhd,bnhd->bmnh", q_n, k_p)
    s_pn = np.einsum("bmhd,bnhd->bmnh", q_p, k_n)
    ti_r = thread_id[:, None]
    ti_c = thread_id[None, :]
    sx = ((ti_r > 0) & (ti_r < ti_c))[None, :, :, None]
    sy = ((ti_c > 0) & (ti_r > ti_c))[None, :, :, None]
    return np.where(sx, s_np, np.where(sy, s_pn, s_pp)).astype(np.float32)


def kernel(qw, kw, token_index, thread_id, _cfg=None):
    qw = np.asarray(qw, dtype=np.float32)
    kw = np.asarray(kw, dtype=np.float32)
    token_index = np.asarray(token_index)
    thread_id = np.asarray(thread_id)
    cfg = _cfg or DEFAULT_CFG

    regimes, ok = _regime_map(thread_id)
    if (
        not ok
        or qw.shape != (B, S, H, D)
        or kw.shape != (B, S, H, D)
        or token_index.shape != (S,)
    ):
        return _reference_fallback(qw, kw, token_index, thread_id)

    qn_blocks = sorted(
        {i for i in range(NB) if any(regimes[i][j] == "np" for j in range(NB))}
    )
    kn_blocks = sorted(
        {j for j in range(NB) if any(regimes[i][j] == "pn" for i in range(NB))}
    )
    if not qn_blocks:
        qn_blocks = [0]
    if not kn_blocks:
        kn_blocks = [0]

    # Device rotation requires every rotated block to share one 128-token
    # index pattern (the [c2|s2|c2] table covers a single period).
    tok_blocks = token_index.reshape(NB, BLK)
    rot_blocks = sorted(set(qn_blocks) | set(kn_blocks))
    uniform = all(
        np.array_equal(tok_blocks[b], tok_blocks[rot_blocks[0]]) for b in rot_blocks
    )
    if not uniform:
        return _reference_fallback(qw, kw, token_index, thread_id)

    rots = _host_rotations(qw, kw, token_index)
    (qp_u, qp_v), (qn_u, qn_v), (kp_u, kp_v), (kn_u, kn_v) = rots
    perm = cfg["perm"]
    qp_a = _to_device_layout(qp_u, qp_v, perm)  # (B,H,2,HALF,S) permuted blocks
    kp_a = _to_device_layout(kp_u, kp_v, perm)

    inv_freq = np.power(
        np.float32(ROPE_BASE),
        (np.arange(HALF, dtype=np.float32) * np.float32(-2.0 / D)),
    )
    theta = (
        token_index[rot_blocks[0] * BLK : (rot_blocks[0] + 1) * BLK]
        .astype(np.float32)[:, None]
        * inv_freq[None, :]
    )
    c2 = np.cos(2.0 * theta).T  # (HALF, BLK)
    s2 = np.sin(2.0 * theta).T
    kt_a = np.ascontiguousarray(
        np.concatenate([c2, s2, -s2, c2], axis=1).astype(np.float16)
    )

    key = (
        tuple(tuple(r) for r in regimes),
        tuple(qn_blocks),
        tuple(kn_blocks),
        str(cfg),
    )
    if key not in _prog_cache:
        _prog_cache[key] = _build_program(regimes, qn_blocks, kn_blocks, cfg)
    nc = _prog_cache[key]

    from concourse.bass_utils import run_bass_kernel_spmd

    in_maps = [{"qp": qp_a[b], "kp": kp_a[b], "kt": kt_a} for b in range(B)]
    trace = bool(int(os.environ.get("KERNEL_TRACE", "0")))
    res = None
    for attempt in range(3):
        try:
            res = run_bass_kernel_spmd(
                nc,
                in_maps,
                core_ids=list(range(N_CORES)),
                trace=trace,
            )
            break
        except Exception:
            # transient NRT/device blips have been observed on otherwise-
            # correct programs; retry.
            if attempt == 2:
                raise
    if res.exec_time_ns is not None:
        print(f"HW exec time: {res.exec_time_ns} ns")
    if res.instructions_and_trace is not None:
        print(f"trace: {res.instructions_and_trace[1]}")

    out_dev = np.stack([res.results[b]["out"] for b in range(B)], axis=0)
    # un-permute output columns: device col position k holds natural block
    # perm[k]
    out = np.empty_like(out_dev)
    for k, b in enumerate(perm):
        out[:, :, b * BLK : (b + 1) * BLK] = out_dev[:, :, k * BLK : (k + 1) * BLK]
    return out.astype(np.float32)


# revision 29
# speedup vs baseline: 1.0513x; 1.0397x over previous
"""Trainium2 Bass kernel for nn_BertWordPair (ragged RoPE pair scores).

Strategy (v3)
-------------
Inputs: qw, kw (B=8, S=768, H=4, D=256) fp32; token_index, thread_id (S,) int32.
Output: (B, S, S, H) fp32 where each (row-block, col-block) pair of the 6x128
thread-block grid uses one of three RoPE sign regimes:
    pp: rope(q,+pos) . rope(k,+pos)
    np: rope(q,-pos) . rope(k,+pos)   (0 < ti_r < ti_c)
    pn: rope(q,+pos) . rope(k,-pos)   (ti_c > 0, ti_r > ti_c)

Batch is sharded across the 8 cores (1 dialogue per core). Host precomputes
the +rotated variants q+, k+ in a de-interleaved (pair-index, token) fp16
layout; q-/k- are derived on device by DVE fp16 rotations
(x- = R(-2theta) x+, one 128-token cos2/sin2 table). Scores are fp16 matmuls
into fp32 PSUM, evacuated (ACT/Pool/DVE) to fp16 stages, streamed out on one
DMA ring (~8.0 MB @ 360 GB/s ~= 22.1 us).

Scheduling tricks:
- Token blocks are stored in DRAM/SBUF in permuted order [1,2,3,4,0,5] so
  the rotation-source blocks 1-4 arrive first and the DVE rotation chain
  (8 x ~1.85 us, the pacing constraint) starts at ~2.6 us.
- Output columns use the same permuted block order (host un-permutes), so
  each half-row's column set is {1,2,3} or {4,0,5}; half-rows are emitted in
  dependency-readiness order (row 0 needs no rotations but the last input
  chunk, so it fills middle slots).
- Dummy matmuls on the rotation table keep the PE p-state ramped while real
  matmul operands are still in flight.
"""

import os

import numpy as np

ROPE_BASE = 10000.0
B, S, H, D = 8, 768, 4, 256
HALF = D // 2  # 128
BLK = 128
NB = S // BLK  # 6
N_CORES = 8

_prog_cache = {}


def _host_rotations(qw, kw, token_index):
    """Return u/v (even/odd) +rotated and -rotated variants, fp32."""
    inv_freq = np.power(
        np.float32(ROPE_BASE),
        (np.arange(HALF, dtype=np.float32) * np.float32(-2.0 / D)),
    )  # (HALF,)
    pos = token_index.astype(np.float32)  # (S,)
    theta = pos[:, None] * inv_freq[None, :]  # (S, HALF)
    cos = np.cos(theta)[None, :, None, :]  # (1,S,1,HALF)
    sin = np.sin(theta)[None, :, None, :]

    out = []
    for x in (qw, kw):
        u = x[..., 0::2]  # (B,S,H,HALF)
        v = x[..., 1::2]
        uc = u * cos
        vs = v * sin
        vc = v * cos
        us = u * sin
        out.append((uc - vs, vc + us))  # positive rotation
        out.append((uc + vs, vc - us))  # negative rotation
    return out  # [(qp_u,qp_v),(qn_u,qn_v),(kp_u,kp_v),(kn_u,kn_v)]


def _to_device_layout(u, v, blocks):
    """(B,S,H,HALF) u/v -> (B, H, 2, HALF, T) fp16 for the given token blocks."""
    cols = np.concatenate([np.arange(b * BLK, (b + 1) * BLK) for b in blocks])
    u = u[:, cols]  # (B,T,H,HALF)
    v = v[:, cols]
    arr = np.stack([u, v], axis=2)  # (B,T,2,H,HALF)
    arr = np.transpose(arr, (0, 3, 2, 4, 1))  # (B,H,2,HALF,T)
    return np.ascontiguousarray(arr.astype(np.float16))


def _regime_map(thread_id):
    """Return (regimes, ok). regimes[i][j] in {'pp','np','pn'} per 128-block."""
    tid = np.asarray(thread_id)
    if tid.shape[0] != S:
        return None, False
    blocks = tid.reshape(NB, BLK)
    if not np.all(blocks == blocks[:, :1]):
        return None, False
    tvals = blocks[:, 0]
    regimes = []
    for i in range(NB):
        row = []
        for j in range(NB):
            ti_r, ti_c = tvals[i], tvals[j]
            if ti_r > 0 and ti_r < ti_c:
                row.append("np")
            elif ti_c > 0 and ti_r > ti_c:
                row.append("pn")
            else:
                row.append("pp")
        regimes.append(row)
    return regimes, True


# ---------------------------------------------------------------------------
# Schedule configuration (tuned against the cost-model simulator).
# ---------------------------------------------------------------------------
DEFAULT_CFG = {
    # token/column block order in DRAM+SBUF (rotation sources first)
    "perm": [1, 2, 3, 4, 0, 5],
    # input DMA chunks over permuted positions [lo, hi), both d-chunks each
    "input_order": [
        ("kp", 0, 2),
        ("kt",),
        ("qp", 0, 2),
        ("kp", 2, 4),
        ("qp", 2, 4),
        ("kp", 4, 6),
        ("qp", 4, 6),
    ],
    # DVE rotation unit order; ("kn", r) = r'th block of kn run
    "rot_order": [
        ("kn", 0),
        ("qn", 0),
        ("kn", 1),
        ("qn", 1),
        ("kn", 2),
        ("qn", 2),
        ("kn", 3),
        ("qn", 3),
    ],
    # output pair groups (row, pairidx) in emission order; pair p covers
    # permuted col positions {2p, 2p+1}; one 2-bank evac + one DMA each
    "groups": [
        (2, 0),
        (1, 0),
        (1, 1),
        (3, 0),
        (4, 0),
        (1, 2),
        (0, 0),
        (0, 1),
        (0, 2),
        (5, 0),
        (5, 2),
        (2, 1),
        (2, 2),
        (4, 1),
        (3, 1),
        (3, 2),
        (5, 1),
        (4, 2),
    ],
    # evac engine per group: A=ACT, P=Pool, V=DVE
    "evac": "APAAPAPAAPAPAPVAVP",
    # rotation units whose final add is folded into the consuming matmuls
    # (4-product accumulation); only sensible for fanout-1 units
    "skip_add": [],
    # PE warmup dummy matmuls before real work (the cost model runs
    # dependency-paced matmuls at full speed, so 0 is best)
    "warmup": 17,
}


def _build_program(regimes, qn_blocks, kn_blocks, cfg):
    import concourse.bass as bass  # noqa: F401
    import concourse.tile as tile
    from concourse import bacc, mybir

    f16 = mybir.dt.float16
    f32 = mybir.dt.float32

    perm = cfg["perm"]
    bpos = {b: i for i, b in enumerate(perm)}
    nqn = len(qn_blocks)
    nkn = len(kn_blocks)
    qn_pos = {b: idx for idx, b in enumerate(qn_blocks)}
    kn_pos = {b: idx for idx, b in enumerate(kn_blocks)}

    nc = bacc.Bacc(None, target_bir_lowering=False)
    qp_d = nc.dram_tensor("qp", [H, 2, HALF, S], f16, kind="ExternalInput")
    kp_d = nc.dram_tensor("kp", [H, 2, HALF, S], f16, kind="ExternalInput")
    # [c2 | s2 | -s2 | c2] over one 128-token period (all rotated blocks share
    # one token pattern; checked on host). tabA=[c2|s2], tabB=[-s2|c2].
    kt_d = nc.dram_tensor("kt", [HALF, 4 * BLK], f16, kind="ExternalInput")
    out_d = nc.dram_tensor("out", [S, S, H], f16, kind="ExternalOutput")

    with tile.TileContext(nc) as tc:
        with (
            tc.tile_pool(name="inp", bufs=1) as inp,
            tc.tile_pool(name="psum", bufs=4, space="PSUM") as pp,
            tc.tile_pool(name="stage", bufs=6) as stp,
            tc.tile_pool(name="rtmp", bufs=4) as rtmp,
        ):
            qp_t = inp.tile([HALF, H * 2 * S], f16, tag="qp")
            kp_t = inp.tile([HALF, H * 2 * S], f16, tag="kp")
            qn_t = inp.tile([HALF, H * 2 * nqn * BLK], f16, tag="qn")
            kn_t = inp.tile([HALF, H * 2 * nkn * BLK], f16, tag="kn")
            kt_t = inp.tile([HALF, 4 * BLK], f16, tag="kt")

            qp_v = qp_t[:].rearrange("p (h c t) -> p h c t", h=H, c=2, t=S)
            kp_v = kp_t[:].rearrange("p (h c t) -> p h c t", h=H, c=2, t=S)
            qp_dv = qp_d[:].rearrange("h c p t -> p h c t")
            kp_dv = kp_d[:].rearrange("h c p t -> p h c t")

            for chunk in cfg["input_order"]:
                if chunk[0] == "kt":
                    nc.sync.dma_start(kt_t[:], kt_d[:])
                else:
                    _, lo, hi = chunk
                    tv = qp_v if chunk[0] == "qp" else kp_v
                    dv = qp_dv if chunk[0] == "qp" else kp_dv
                    nc.sync.dma_start(
                        tv[:, :, :, lo * BLK : hi * BLK],
                        dv[:, :, :, lo * BLK : hi * BLK],
                    )

            # PE warmup: dummy matmuls on a memset tile (no input dependency)
            # start the tensor engine's p-state ramp clock immediately, so
            # real matmuls run at full speed. They write group 0's bank tile,
            # whose real matmuls overwrite it afterwards.
            warm = pp.tile([BLK, 2 * BLK * H], f32, tag="bank")
            if cfg["warmup"]:
                wsrc = inp.tile([BLK, 2 * BLK], f16, tag="wsrc")
                nc.vector.memset(wsrc[:], 0.0)
                for w in range(cfg["warmup"]):
                    nc.tensor.matmul(
                        warm[:, (w % 8) * BLK : (w % 8) * BLK + BLK],
                        wsrc[:, 0:BLK],
                        wsrc[:, BLK : 2 * BLK],
                        start=True,
                        stop=True,
                    )

            tabA = kt_t[:, 0 : 2 * BLK].rearrange("p (c t) -> p c t", c=2)
            tabB = kt_t[:, 2 * BLK : 4 * BLK].rearrange("p (c t) -> p c t", c=2)
            tabA4 = tabA.copy()
            tabA4.ap = tabA4.ap[:1] + [[0, H]] + tabA4.ap[1:]
            tabB4 = tabB.copy()
            tabB4.ap = tabB4.ap[:1] + [[0, H]] + tabB4.ap[1:]

            skip_add = {tuple(u) for u in cfg.get("skip_add", [])}
            zsaved = {}

            def emit_rotation(unit):
                # x- = R(-2theta) x+ for one 128-token block, all heads:
                #   u- = u*c2 + v*s2 ; v- = v*c2 - u*s2
                # X  = (u,v)*[c2|s2]  -> u- = X.lo  + X.hi
                # Y' = (u,v)*[-s2|c2] -> v- = Y'.lo + Y'.hi
                # One fused add produces both: dst[c=xy] = Z[xy].lo + Z[xy].hi
                # For skip_add units the add is folded into the consuming
                # matmuls (4-product accumulation), so only Z is produced.
                kind, ridx = unit
                src_t = qp_t if kind == "qn" else kp_t
                dst_t = qn_t if kind == "qn" else kn_t
                b = (qn_blocks if kind == "qn" else kn_blocks)[ridx]
                nrun = nqn if kind == "qn" else nkn
                p0 = bpos[b] * BLK
                src = (
                    src_t[:]
                    .rearrange("p (h c t) -> p h c t", h=H, c=2, t=S)[
                        :, :, :, p0 : p0 + BLK
                    ]
                )  # (p, h, 2, 128)
                if unit in skip_add:
                    z = rtmp.tile([HALF, 2 * H * 2 * BLK], f16, tag="zkeep")
                else:
                    z = rtmp.tile([HALF, 2 * H * 2 * BLK], f16, tag="z")
                z_v = z[:].rearrange("p (xy h c t) -> p xy h c t", xy=2, h=H, c=2)
                nc.vector.tensor_mul(z_v[:, 0], src, tabA4)
                nc.vector.tensor_mul(z_v[:, 1], src, tabB4)
                if unit in skip_add:
                    zsaved[unit] = z_v
                    return
                dst = (
                    dst_t[:]
                    .rearrange("p (h c t) -> p c h t", h=H, c=2, t=nrun * BLK)[
                        :, :, :, ridx * BLK : (ridx + 1) * BLK
                    ]
                )  # (p, c, h, 128) with c outermost
                nc.vector.tensor_add(dst, z_v[:, :, :, 0], z_v[:, :, :, 1])

            for unit in cfg["rot_order"]:
                kind, ridx = unit
                if ridx < (nqn if kind == "qn" else nkn):
                    emit_rotation(unit)

            def lhs_slice(variant, h, c, blk):
                if variant == "p":
                    return qp_t[:, (h * 2 + c) * S + bpos[blk] * BLK :][:, :BLK]
                return qn_t[:, (h * 2 + c) * (nqn * BLK) + qn_pos[blk] * BLK :][:, :BLK]

            def rhs_slice(variant, h, c, blk):
                if variant == "p":
                    return kp_t[:, (h * 2 + c) * S + bpos[blk] * BLK :][:, :BLK]
                return kn_t[:, (h * 2 + c) * (nkn * BLK) + kn_pos[blk] * BLK :][:, :BLK]

            for gidx, (i, pair) in enumerate(cfg["groups"]):
                stage = stp.tile([BLK, 2 * BLK * H], f16, tag="pair")
                bank2 = warm if gidx == 0 else pp.tile(
                    [BLK, 2 * BLK * H], f32, tag="bank"
                )
                for idx in range(2):
                    jj = pair * 2 + idx
                    j = perm[jj]
                    reg = regimes[i][j]
                    qv = "n" if reg == "np" else "p"
                    kv = "n" if reg == "pn" else "p"
                    zq = zsaved.get(("qn", qn_pos.get(i))) if reg == "np" else None
                    zk = zsaved.get(("kn", kn_pos.get(j))) if reg == "pn" else None
                    for h in range(H):
                        if zq is not None:
                            # qn_i folded: qn_e = X.lo+X.hi, qn_o = Y.lo+Y.hi
                            mms = [
                                (zq[:, 0, h, 0], rhs_slice("p", h, 0, j)),
                                (zq[:, 0, h, 1], rhs_slice("p", h, 0, j)),
                                (zq[:, 1, h, 0], rhs_slice("p", h, 1, j)),
                                (zq[:, 1, h, 1], rhs_slice("p", h, 1, j)),
                            ]
                        elif zk is not None:
                            mms = [
                                (lhs_slice("p", h, 0, i), zk[:, 0, h, 0]),
                                (lhs_slice("p", h, 0, i), zk[:, 0, h, 1]),
                                (lhs_slice("p", h, 1, i), zk[:, 1, h, 0]),
                                (lhs_slice("p", h, 1, i), zk[:, 1, h, 1]),
                            ]
                        else:
                            mms = [
                                (lhs_slice(qv, h, 0, i), rhs_slice(kv, h, 0, j)),
                                (lhs_slice(qv, h, 1, i), rhs_slice(kv, h, 1, j)),
                            ]
                        for m, (lhs, rhs) in enumerate(mms):
                            nc.tensor.matmul(
                                bank2[
                                    :,
                                    (idx * H + h) * BLK : (idx * H + h + 1) * BLK,
                                ],
                                lhs,
                                rhs,
                                start=(h == 0 and m == 0),
                                stop=(h == H - 1 and m == len(mms) - 1),
                            )
                # one 2-bank head-interleaving evac: (p, (jj h n)) fp32 ->
                # (p, (jj n h)) fp16
                dst_blk = stage[:].rearrange("p (jj n h) -> p jj n h", jj=2, h=H)
                src_blk = bank2[:].rearrange("p (jj h n) -> p jj n h", jj=2, h=H)
                eng = cfg["evac"][gidx]
                if eng == "A":
                    nc.scalar.copy(dst_blk, src_blk)
                elif eng == "P":
                    nc.gpsimd.tensor_copy(dst_blk, src_blk)
                else:
                    nc.vector.tensor_copy(dst_blk, src_blk)
                nc.sync.dma_start(
                    out_d[
                        i * BLK : (i + 1) * BLK,
                        pair * (2 * BLK) : (pair + 1) * (2 * BLK),
                    ].rearrange("p n h -> p (n h)"),
                    stage[:],
                )
    nc.finalize()
    return nc


def _reference_fallback(qw, kw, token_index, thread_id):
    """Pure numpy fallback for unexpected block structure."""
    rots = _host_rotations(qw, kw, token_index)
    (qp_u, qp_v), (qn_u, qn_v), (kp_u, kp_v), (kn_u, kn_v) = rots

    def interleave(u, v):
        x = np.empty(u.shape[:-1] + (D,), dtype=np.float32)
        x[..., 0::2] = u
        x[..., 1::2] = v
        return x

    q_p = interleave(qp_u, qp_v)
    q_n = interleave(qn_u, qn_v)
    k_p = interleave(kp_u, kp_v)
    k_n = interleave(kn_u, kn_v)
    s_pp = np.einsum("bmhd,bnhd->bmnh", q_p, k_p)
    s_np = np.einsum("bmhd,bnhd->bmnh", q_n, k_p)
    s_pn = np.einsum("bmhd,bnhd->bmnh", q_p, k_n)
    ti_r = thread_id[:, None]
    ti_c = thread_id[None, :]
    sx = ((ti_r > 0) & (ti_r < ti_c))[None, :, :, None]
    sy = ((ti_c > 0) & (ti_r > ti_c))[None, :, :, None]
    return np.where(sx, s_np, np.where(sy, s_pn, s_pp)).astype(np.float32)


def kernel(qw, kw, token_index, thread_id, _cfg=None):
    qw = np.asarray(qw, dtype=np.float32)
    kw = np.asarray(kw, dtype=np.float32)
    token_index = np.asarray(token_index)
    thread_id = np.asarray(thread_id)
    cfg = _cfg or DEFAULT_CFG

    regimes, ok = _regime_map(thread_id)
    if (
        not ok
        or qw.shape != (B, S, H, D)
        or kw.shape != (B, S, H, D)
        or token_index.shape != (S,)
    ):
        return _reference_fallback(qw, kw, token_index, thread_id)

    qn_blocks = sorted(
        {i for i in range(NB) if any(regimes[i][j] == "np" for j in range(NB))}
    )
    kn_blocks = sorted(
        {j for j in range(NB) if any(regimes[i][j] == "pn" for i in range(NB))}
    )
    if not qn_blocks:
        qn_blocks = [0]
    if not kn_blocks:
        kn_blocks = [0]

    # Device rotation requires every rotated block to share one 128-token
    # index pattern (the [c2|s2|c2] table covers a single period).
    tok_blocks = token_index.reshape(NB, BLK)
    rot_blocks = sorted(set(qn_blocks) | set(kn_blocks))
    uniform = all(
        np.array_equal(tok_blocks[b], tok_blocks[rot_blocks[0]]) for b in rot_blocks
    )
    if not uniform:
        return _reference_fallback(qw, kw, token_index, thread_id)

    rots = _host_rotations(qw, kw, token_index)
    (qp_u, qp_v), (qn_u, qn_v), (kp_u, kp_v), (kn_u, kn_v) = rots
    perm = cfg["perm"]
    qp_a = _to_device_layout(qp_u, qp_v, perm)  # (B,H,2,HALF,S) permuted blocks
    kp_a = _to_device_layout(kp_u, kp_v, perm)

    inv_freq = np.power(
        np.float32(ROPE_BASE),
        (np.arange(HALF, dtype=np.float32) * np.float32(-2.0 / D)),
    )
    theta = (
        token_index[rot_blocks[0] * BLK : (rot_blocks[0] + 1) * BLK]
        .astype(np.float32)[:, None]
        * inv_freq[None, :]
    )
    c2 = np.cos(2.0 * theta).T  # (HALF, BLK)
    s2 = np.sin(2.0 * theta).T
    kt_a = np.ascontiguousarray(
        np.concatenate([c2, s2, -s2, c2], axis=1).astype(np.float16)
    )

    key = (
        tuple(tuple(r) for r in regimes),
        tuple(qn_blocks),
        tuple(kn_blocks),
        str(cfg),
    )
    if key not in _prog_cache:
        _prog_cache[key] = _build_program(regimes, qn_blocks, kn_blocks, cfg)
    nc = _prog_cache[key]

    from concourse.bass_utils import run_bass_kernel_spmd

    in_maps = [{"qp": qp_a[b], "kp": kp_a[b], "kt": kt_a} for b in range(B)]
    trace = bool(int(os.environ.get("KERNEL_TRACE", "0")))
    res = None
    for attempt in range(3):
        try:
            res = run_bass_kernel_spmd(
                nc,
                in_maps,
                core_ids=list(range(N_CORES)),
                trace=trace,
            )
            break
        except Exception:
            # transient NRT/device blips have been observed on otherwise-
            # correct programs; retry.
            if attempt == 2:
                raise
    if res.exec_time_ns is not None:
        print(f"HW exec time: {res.exec_time_ns} ns")
    if res.instructions_and_trace is not None:
        print(f"trace: {res.instructions_and_trace[1]}")

    out_dev = np.stack([res.results[b]["out"] for b in range(B)], axis=0)
    # un-permute output columns: device col position k holds natural block
    # perm[k]
    out = np.empty_like(out_dev)
    for k, b in enumerate(perm):
        out[:, :, b * BLK : (b + 1) * BLK] = out_dev[:, :, k * BLK : (k + 1) * BLK]
    return out.astype(np.float32)
